# revision 36
# baseline (speedup 1.0000x reference)
"""Trainium2 Bass kernel for nn_DecoderRNN (2-layer LSTM + attention decoder).

Strategy (8 NeuronCores, no collectives):
  - Phase A: the sequential LSTM recurrence is replicated on every core
    (PE cost is weight-streaming bound and independent of batch, so
    replication is free).  Each step additionally projects h2 through a
    per-core one-hot selection matrix (a tiny PE matmul) so every core
    accumulates the transposed h2 history for ITS OWN 8 batches.
  - Phase B: attention + concat projection run on the core's 8 batches
    only (1/8 of the old work), then the output projection computes the
    FULL 32000-vocab logits for the core's 256 rows, streaming W_out
    from HBM in 2048-column chunks.  log_softmax is then entirely
    core-local: no AllReduce, no cross-core synchronization at all.
Row convention: core c owns batches [8c, 8c+8); local row r = j*T + t,
so concatenating the 8 core outputs on axis 0 gives b-major [B*T, V].
"""

import os as _os_env
_os_env.environ.setdefault("JAX_COMPILATION_CACHE_DIR", "/root/jaxcache")
_os_env.environ.setdefault("JAX_PERSISTENT_CACHE_MIN_COMPILE_TIME_SECS", "1")
_os_env.environ.setdefault("JAX_PERSISTENT_CACHE_MIN_ENTRY_SIZE_BYTES", "0")

import numpy as np

import concourse.bass as bass
import concourse.mybir as mybir
import concourse.tile as tile
from concourse import bacc
from concourse.bass_utils import run_bass_kernel_spmd
from concourse.masks import make_identity
from concourse.alu_op_type import AluOpType

F32 = mybir.dt.float32
F32R = mybir.dt.float32r
BF16 = mybir.dt.bfloat16
FP8 = mybir.dt.float8e4
PM_DR = mybir.MatmulPerfMode.DoubleRow
AF = mybir.ActivationFunctionType

# fp8 scaling: weights and (transposed) hidden states are stored as
# x*SCALE in e4m3; a gates psum therefore carries SCALE^2 * gates and
# the activation unscales with scale=INV_GS.
SCALE = 64.0
INV_GS = 1.0 / (SCALE * SCALE)

NCORES = 8
B = 64
H = 512
E = 512
L = 49
V_FULL = 32000
BL = B // NCORES  # local batches per core (8)


def build_nc2(T=32, use_b1=False, use_b2=False, use_bout=False,
              _debug_stop=None, lag=3):
    """v3: software-pipelined recurrence + fully core-local phase B.

    - cell1 input projections (X1 = emb @ W_ih1.T) precomputed via a
      t-pair-packed GEMM (M=128), interleaved into the loop as PE filler.
    - gate order permuted host-side to (g, i, f, o) so the gate
      nonlinearities collapse to 1 tanh + 1 sigmoid call per cell.
    - gates PSUM [128, 2048]: cell1 on partitions 0-63, cell2 (lag steps
      behind) on partitions 64-127.
    - per-step h2 selection matmul stashes the core's 8 batches into
      h2lS [H, BL*T] (k-major transposed layout) for phase B.
    """
    assert not use_bout, "non-zero b_out not supported in local phase B"
    R_l = BL * T          # local rows (256)
    NRTl = R_l // 128     # local row tiles (2)
    K_E = E // 128  # 4
    K_H = H // 128  # 4
    K2H = 2 * H // 128
    NP = T // 2  # t-pairs
    G4 = 4 * H   # 2048
    V = V_FULL
    VCH = 2048
    NCH = (V + VCH - 1) // VCH  # 16

    nc = bacc.Bacc("TRN2", target_bir_lowering=False, num_devices=NCORES)

    embT_d = nc.dram_tensor("embT", [E, T * B], FP8, kind="ExternalInput")
    wih1_d = nc.dram_tensor("wih1T", [E, G4], FP8, kind="ExternalInput")
    whh1_d = nc.dram_tensor("whh1T", [H, G4], FP8, kind="ExternalInput")
    wih2_d = nc.dram_tensor("wih2T", [H, G4], FP8, kind="ExternalInput")
    whh2_d = nc.dram_tensor("whh2T", [H, G4], FP8, kind="ExternalInput")
    wcat_d = nc.dram_tensor("wcatT2", [2 * H, H], BF16, kind="ExternalInput")
    ftl_d = nc.dram_tensor("FTl", [BL, H, L], BF16, kind="ExternalInput")
    fl_d = nc.dram_tensor("Fl", [BL, L, H], BF16, kind="ExternalInput")
    wout_d = nc.dram_tensor("woutF", [H, V], FP8, kind="ExternalInput")
    sel_d = nc.dram_tensor("sel", [64, BL], BF16, kind="ExternalInput")
    bcat_d = nc.dram_tensor("bcat", [H], F32, kind="ExternalInput")
    b1_d = nc.dram_tensor("b1", [G4], F32, kind="ExternalInput")
    b2_d = nc.dram_tensor("b2", [G4], F32, kind="ExternalInput")
    out_d = nc.dram_tensor("out", [R_l, V], BF16, kind="ExternalOutput")

    def load_kmaj(dst_tile, src_ap, K, N):
        src = src_ap.rearrange("(k p) n -> p k n", p=128)
        dst = dst_tile[:].rearrange("p (k n) -> p k n", n=N)
        nc.sync.dma_start(dst, src)

    with tile.TileContext(nc) as tc:
        with tc.tile_pool(name="const", bufs=1) as constp, \
             tc.tile_pool(name="persist", bufs=1) as pp:
            ident = constp.tile([128, 128], BF16, tag="ident")
            make_identity(nc, ident[:])
            bcat_sb = constp.tile([128, K_H], F32, tag="bcat")
            nc.sync.dma_start(
                bcat_sb[:],
                bcat_d.ap().rearrange("(k p) -> p k", p=128),
            )
            selsb = constp.tile([128, BL], BF16, tag="selsb")
            nc.sync.dma_start(selsb[0:64, :], sel_d.ap())
            nc.sync.dma_start(selsb[64:128, :], sel_d.ap())

            # transposed per-core h2 history [H, BL*T] (k-major)
            h2lS = pp.tile([128, K_H * R_l], BF16, tag="h2lS")
            zc = pp.tile([128, 2 * NCH], F32, tag="zc")

            # ================= PHASE A =================
            with tc.tile_pool(name="wts", bufs=1) as wp, \
                 tc.tile_pool(name="xemb", bufs=4) as xp, \
                 tc.tile_pool(name="acts", bufs=3) as apool, \
                 tc.tile_pool(name="st", bufs=4) as sp, \
                 tc.tile_pool(name="ew", bufs=4) as ewp, \
                 tc.tile_pool(name="ghp", bufs=3, space="PSUM") as gps, \
                 tc.tile_pool(name="t2psum", bufs=1, space="PSUM") as t2ps, \
                 tc.tile_pool(name="tpsum", bufs=1, space="PSUM") as tps:

                wih1 = wp.tile([128, K_E * G4], FP8, tag="wih1")
                whh1 = wp.tile([128, K_H * G4], FP8, tag="whh1")
                wih2 = wp.tile([128, K_H * G4], FP8, tag="wih2")
                whh2 = wp.tile([128, K_H * G4], FP8, tag="whh2")
                load_kmaj(wih1, wih1_d.ap(), K_E, G4)
                load_kmaj(whh1, whh1_d.ap(), K_H, G4)
                load_kmaj(wih2, wih2_d.ap(), K_H, G4)
                load_kmaj(whh2, whh2_d.ap(), K_H, G4)
                wihv = {1: wih1[:].rearrange("p (k g) -> p k g", g=G4),
                        2: wih2[:].rearrange("p (k g) -> p k g", g=G4)}
                whhv = {1: whh1[:].rearrange("p (k g) -> p k g", g=G4),
                        2: whh2[:].rearrange("p (k g) -> p k g", g=G4)}

                # biases (pre-scaled by SCALE^2 host-side) injected via a
                # plain identity matmul on partitions 0-63
                bbc = {1: None, 2: None}
                if use_b1:
                    b1bc = constp.tile([64, G4], BF16, tag="b1bc")
                    nc.sync.dma_start(
                        b1bc[:], b1_d.ap().unsqueeze(0).partition_broadcast(64)
                    )
                    bbc[1] = b1bc
                if use_b2:
                    b2bc = constp.tile([64, G4], BF16, tag="b2bc")
                    nc.sync.dma_start(
                        b2bc[:], b2_d.ap().unsqueeze(0).partition_broadcast(64)
                    )
                    bbc[2] = b2bc

                def emit_xemb(t):
                    xe = xp.tile([128, K_E * 64], FP8, tag="xe", name=f"xe{t}")
                    nc.sync.dma_start(
                        xe[:].rearrange("p (k n) -> p k n", n=64),
                        embT_d.ap()[:, t * 64:(t + 1) * 64].rearrange(
                            "(k p) n -> p k n", p=128),
                    )
                    return xe

                def emit_cell_mms(cell, t, gA, gB, xstat, hstat):
                    """All-DoubleRow gates for one cell into two [64, 1024]
                    psum halves (DR dst must sit on partitions 0-63)."""
                    stats = [(xstat, wihv[cell])]
                    if hstat is not None:
                        stats.append((hstat, whhv[cell]))
                    bias = bbc[cell]
                    nmm = 2 * len(stats) + (1 if bias is not None else 0)
                    for half, gg in ((0, gA), (1, gB)):
                        for n2 in range(2):
                            col = half * 1024 + n2 * 512
                            out = gg[:, n2 * 512:(n2 + 1) * 512]
                            i = 0
                            if bias is not None:
                                nc.tensor.matmul(
                                    out, ident[0:64, 0:64],
                                    bias[:, col:col + 512],
                                    start=True, stop=(nmm == 1),
                                )
                                i = 1
                            for sv, wv in stats:
                                s3 = sv.rearrange("p (k b) -> p k b", b=64)
                                for kp in (0, 2):
                                    nc.tensor.matmul(
                                        out,
                                        s3[:, kp:kp + 2, :],
                                        wv[:, kp:kp + 2, col:col + 512],
                                        start=(i == 0), stop=(i == nmm - 1),
                                        perf_mode=PM_DR,
                                    )
                                    i += 1

                def emit_tail(cell, t, gA, gB, c_prev, it):
                    """it = shared per-iteration scratch dict.  Both cells
                    live on partitions 0-63."""
                    idsl = ident[0:64, 0:64]
                    a = it[f"a{cell}"]
                    # gate order (g, i | f, o) across the two psum halves
                    nc.scalar.activation(a[:, 0:512], gA[:, 0:512],
                                         AF.Tanh, scale=INV_GS)
                    nc.scalar.activation(a[:, 512:1024], gA[:, 512:1024],
                                         AF.Sigmoid, scale=INV_GS)
                    nc.scalar.activation(a[:, 1024:2048], gB[:],
                                         AF.Sigmoid, scale=INV_GS)
                    tg = a[:, 0:512]
                    si = a[:, 512:1024]
                    sf = a[:, 1024:1536]
                    so = a[:, 1536:2048]
                    cn = it[f"c{cell}"]
                    if c_prev is None:
                        nc.vector.tensor_tensor(cn[:], si, tg, AluOpType.mult)
                    else:
                        tmp = it[f"tmp{cell}"]
                        nc.vector.tensor_tensor(tmp[:], si, tg, AluOpType.mult)
                        nc.vector.tensor_tensor(cn[:], sf, c_prev[:],
                                                AluOpType.mult)
                        nc.vector.tensor_tensor(cn[:], cn[:], tmp[:],
                                                AluOpType.add)
                    tct = it[f"tct{cell}"]
                    nc.scalar.activation(tct[:], cn[:], AF.Tanh)
                    hn = it[f"hn{cell}"]
                    nc.vector.tensor_tensor(hn[:], so, tct[:], AluOpType.mult)
                    tp = it["tp"][:, (cell - 1) * 256:cell * 256]
                    for k in range(K_H):
                        nc.tensor.transpose(
                            tp[:, k * 64:(k + 1) * 64],
                            hn[:, k * 128:(k + 1) * 128],
                            idsl,
                        )
                    hT = it["hT"][:, (cell - 1) * 256:cell * 256]
                    nc.vector.tensor_scalar_mul(hT, tp, SCALE)
                    if cell == 2:
                        # stash the core's BL batches of h2 (transposed) via
                        # the per-core one-hot selection matmul
                        t2p = t2ps.tile([128, K_H * BL], F32, tag="t2p",
                                        name=f"t2p{t}")
                        for k in range(K_H):
                            nc.tensor.matmul(
                                t2p[:, k * BL:(k + 1) * BL],
                                hn[:, k * 128:(k + 1) * 128],
                                selsb[0:64, :],
                                start=True, stop=True,
                            )
                        nc.vector.tensor_copy(
                            h2lS[:].rearrange("p (k r) -> p k r",
                                              k=K_H)[:, :, t::T],
                            t2p[:].rearrange("p (k j) -> p k j", j=BL),
                        )
                    return cn, hT

                def iter_tiles(i, cells=(1, 2)):
                    d = {
                        "hT": sp.tile([128, 512], FP8, tag="hT",
                                      name=f"hT_{i}"),
                        "tp": tps.tile([128, 512], BF16, tag="tp",
                                       name=f"tp_{i}"),
                    }
                    for cell in cells:
                        d[f"a{cell}"] = apool.tile(
                            [64, G4], BF16, tag=f"a{cell}",
                            name=f"a{cell}_{i}")
                        d[f"c{cell}"] = sp.tile(
                            [64, 512], F32, tag=f"c{cell}", name=f"c{cell}_{i}")
                        d[f"tmp{cell}"] = ewp.tile(
                            [64, 512], F32, tag=f"tmp{cell}",
                            name=f"tmp{cell}_{i}")
                        d[f"tct{cell}"] = ewp.tile(
                            [64, 512], BF16, tag=f"tct{cell}",
                            name=f"tct{cell}_{i}")
                        d[f"hn{cell}"] = ewp.tile(
                            [64, 512], BF16, tag=f"hn{cell}",
                            name=f"hn{cell}_{i}")
                    return d

                def g(nm):
                    return gps.tile([64, 1024], F32, tag="gh", name=nm)

                # software pipeline: iteration i emits cell1(i+1) and
                # cell2(i+1-lag); cell2's MMs fill the PE during cell1's
                # ACT/DVE tail (inputs all ready).
                LAG = lag
                xes = {t: emit_xemb(t) for t in range(min(3, T))}
                it = iter_tiles(-1, (1,))
                gA, gB = g("g0A"), g("g0B")
                emit_cell_mms(1, 0, gA, gB, xes[0][:], None)
                c1, h1T = emit_tail(1, 0, gA, gB, None, it)
                c2 = h2T = None
                h1Ts = {0: h1T}  # keep h1T(t) alive until cell2(t) consumes

                for i in range(T + LAG - 1):
                    t1 = i + 1          # cell1 step emitted this iteration
                    t2 = i + 1 - LAG    # cell2 step emitted this iteration
                    cells = tuple(c for c, on in ((1, t1 < T),
                                                  (2, 0 <= t2 < T)) if on)
                    it = iter_tiles(i, cells)
                    nh1T = nc1 = None
                    if t1 < T:
                        gA1, gB1 = g(f"g{i}a"), g(f"g{i}b")
                        emit_cell_mms(1, t1, gA1, gB1, xes[t1][:], h1T)
                    if 0 <= t2 < T:
                        gA2, gB2 = g(f"g{i}c"), g(f"g{i}d")
                        emit_cell_mms(2, t2, gA2, gB2, h1Ts[t2], h2T)
                    if t1 + 2 < T:
                        xes[t1 + 2] = emit_xemb(t1 + 2)
                    if t1 < T:
                        nc1, nh1T = emit_tail(1, t1, gA1, gB1, c1, it)
                    if 0 <= t2 < T:
                        c2, h2T = emit_tail(2, t2, gA2, gB2, c2, it)
                        h1Ts.pop(t2, None)
                        xes.pop(t2, None)
                    if t1 < T:
                        c1, h1T = nc1, nh1T
                        h1Ts[t1] = h1T

            if _debug_stop == "A":
                nc.gpsimd.dma_start(out_d.ap()[0:128, 0:K_H * R_l],
                                    h2lS[:, 0:K_H * R_l])
            else:
                _phase_b_local(nc, tc, constp, T, R_l, NRTl, K_H, K2H,
                               NCH, VCH, V, ident, bcat_sb, h2lS, zc,
                               wcat_d, ftl_d, fl_d, wout_d, out_d,
                               load_kmaj)

    nc.finalize()
    return nc


def _phase_b_local(nc, tc, constp, T, R_l, NRTl, K_H, K2H, NCH, VCH, V,
                   ident, bcat_sb, h2lS, zc, wcat_d, ftl_d, fl_d, wout_d,
                   out_d, load_kmaj):
    NG = BL // 4  # score groups of 4 batches (2)
    LN2 = 0.6931471805599453

    with tc.tile_pool(name="bloc", bufs=1) as blp:
        concTl = blp.tile([128, K_H * R_l], BF16, tag="concTl")

        # ---- attention + context + concat on the core's BL batches ----
        with tc.tile_pool(name="wcatp", bufs=1) as wcp, \
             tc.tile_pool(name="attn", bufs=1) as ap_, \
             tc.tile_pool(name="fstream", bufs=2) as fsp:
            wcat = wcp.tile([128, K2H * H], BF16, tag="wcat")
            load_kmaj(wcat, wcat_d.ap(), K2H, H)
            ctxTl = ap_.tile([128, K_H * R_l], BF16, tag="ctxTl")
            expS = ap_.tile([128, NG * L], F32, tag="expS")
            Zt = ap_.tile([128, NG], F32, tag="Zt")
            Rt = ap_.tile([128, NG], F32, tag="Rt")
            attnT = ap_.tile([L, R_l], BF16, tag="attnT")

            with tc.tile_pool(name="scps", bufs=2, space="PSUM") as scps, \
                 tc.tile_pool(name="atps", bufs=1, space="PSUM") as atps:
                atp = atps.tile([L, R_l], BF16, tag="atp")
                for g in range(NG):
                    ftb = fsp.tile([128, 4 * K_H * L], BF16, tag="ftb",
                                   name=f"ftb{g}")
                    nc.sync.dma_start(
                        ftb[:].rearrange("p (b k l) -> p b k l", l=L, k=K_H),
                        ftl_d.ap()[4 * g:4 * (g + 1)].rearrange(
                            "b (k p) l -> p b k l", p=128),
                    )
                    ftb4d = ftb[:].rearrange("p (b k l) -> p b k l",
                                             l=L, k=K_H)
                    scp = scps.tile([128, 4 * L], F32, tag="sc", name=f"sc{g}")
                    for k in range(K_H):
                        nc.tensor.matmul(
                            scp[:],
                            h2lS[:, k * R_l + g * 128:k * R_l + (g + 1) * 128],
                            ftb4d[:, :, k, :],
                            start=(k == 0), stop=(k == K_H - 1),
                        )
                    for j in range(4):
                        nc.scalar.activation(
                            expS[j * T:(j + 1) * T, g * L:(g + 1) * L],
                            scp[j * T:(j + 1) * T, j * L:(j + 1) * L],
                            AF.Exp,
                        )
                nc.vector.tensor_reduce(
                    Zt[:],
                    expS[:].rearrange("t (g l) -> t g l", l=L),
                    mybir.AxisListType.X, AluOpType.add,
                )
                nc.vector.reciprocal(Rt[:], Zt[:])
                attnN = ap_.tile([128, NG * L], BF16, tag="attnN")
                for g in range(NG):
                    nc.vector.tensor_scalar_mul(
                        attnN[:, g * L:(g + 1) * L],
                        expS[:, g * L:(g + 1) * L],
                        Rt[:, g:g + 1],
                    )
                    nc.tensor.transpose(
                        atp[:L, g * 128:(g + 1) * 128],
                        attnN[:, g * L:(g + 1) * L],
                        ident[:, :],
                    )
                nc.vector.tensor_copy(attnT[:], atp[:])

            # context: one group of BL batches
            with tc.tile_pool(name="ctxps", bufs=4, space="PSUM") as cps:
                ctls = [
                    cps.tile([128, BL * T], F32, tag="ctx", name=f"ctx{m}")
                    for m in range(K_H)
                ]
                fb8 = fsp.tile([L, BL * H], BF16, tag="fb8", name="fb8")
                nc.sync.dma_start(
                    fb8[:].rearrange("l (b h) -> l b h", h=H),
                    fl_d.ap().rearrange("b l h -> l b h"),
                )
                for j in range(BL):
                    fb = fb8[:, j * H:(j + 1) * H]
                    for m in range(K_H):
                        nc.tensor.matmul(
                            ctls[m][:, j * T:(j + 1) * T],
                            fb[:, m * 128:(m + 1) * 128],
                            attnT[:, j * T:(j + 1) * T],
                            start=True, stop=True,
                        )
                for m in range(K_H):
                    nc.vector.tensor_copy(
                        ctxTl[:, m * R_l:(m + 1) * R_l], ctls[m][:])

            # concat projection: concTl = tanh(Wcat @ [ctx; h2] + bcat)
            with tc.tile_pool(name="ccps", bufs=2, space="PSUM") as ccps:
                for m in range(K_H):
                    ccp = ccps.tile([128, R_l], F32, tag="cc", name=f"cc{m}")
                    for k in range(K2H):
                        rhs = (ctxTl if k < K_H else h2lS)
                        kk = k if k < K_H else k - K_H
                        nc.tensor.matmul(
                            ccp[:],
                            wcat[:, k * H + m * 128:k * H + (m + 1) * 128],
                            rhs[:, kk * R_l:kk * R_l + R_l],
                            start=(k == 0), stop=(k == K2H - 1),
                        )
                    nc.scalar.activation(
                        concTl[:, m * R_l:(m + 1) * R_l], ccp[:], AF.Tanh,
                        bias=bcat_sb[:, m:m + 1],
                    )

        # fp8 copy of concTl (scaled) for the DoubleRow output projection
        concT8 = blp.tile([128, K_H * R_l], FP8, tag="concT8")
        nc.vector.tensor_scalar_mul(concT8[:], concTl[:], SCALE)

        # ---- B2: full-vocab logits for the core's rows, streamed W_out ----
        with tc.tile_pool(name="vsbp", bufs=1) as vp, \
             tc.tile_pool(name="scr", bufs=2) as scrp:

            vsb = [vp.tile([128, V], BF16, tag=f"vsb{tl}", name=f"vsb{tl}")
                   for tl in range(NRTl)]

            with tc.tile_pool(name="wsp", bufs=3) as wsp, \
                 tc.tile_pool(name="osb", bufs=2) as osbp, \
                 tc.tile_pool(name="osub", bufs=6) as osp, \
                 tc.tile_pool(name="lgps", bufs=2, space="PSUM") as lgps:
                wos = {}

                def emit_wo(tl, ci):
                    cw = min(VCH, V - ci * VCH)
                    wo = wsp.tile([128, K_H * VCH], FP8, tag="wo",
                                  name=f"wo{tl}_{ci}")
                    nc.sync.dma_start(
                        wo[:].rearrange("p (k n) -> p k n", n=VCH)[:, :, :cw],
                        wout_d.ap()[:, ci * VCH:ci * VCH + cw].rearrange(
                            "(k p) n -> p k n", p=128),
                    )
                    wos[(tl, ci)] = wo

                c8v = concT8[:].rearrange("p (k r) -> p k r", r=R_l)

                def emit_epilogue(tl):
                    """-ln(Z) + subtract + store for one row tile (its Z is
                    complete; overlaps the other tile's matmul sweep)."""
                    Z1 = scrp.tile([128, 1], F32, tag="Z2", name=f"Z{tl}")
                    nc.vector.tensor_reduce(
                        Z1[:],
                        zc[:, tl * NCH:(tl + 1) * NCH].unsqueeze(1),
                        mybir.AxisListType.X, AluOpType.add,
                    )
                    zi = scrp.tile([128, 1], F32, tag="zi", name=f"zi{tl}")
                    nc.vector.tensor_copy(zi[:], Z1[:].bitcast(mybir.dt.int32))
                    m = scrp.tile([128, 1], F32, tag="nm", name=f"m0_{tl}")
                    nc.vector.tensor_scalar(
                        m[:], zi[:], -LN2 / (1 << 23), 127.0 * LN2 - 0.0299,
                        AluOpType.mult, AluOpType.add,
                    )
                    for it in range(2):
                        e = scrp.tile([128, 1], F32, tag="ne",
                                      name=f"e{tl}_{it}")
                        nc.scalar.activation(e[:], m[:], AF.Exp)
                        w = scrp.tile([128, 1], F32, tag="nw",
                                      name=f"w{tl}_{it}")
                        nc.vector.tensor_tensor(w[:], e[:], Z1[:],
                                                AluOpType.mult)
                        m2 = scrp.tile([128, 1], F32, tag="nm",
                                       name=f"m{tl}_{it + 1}")
                        nc.vector.tensor_tensor(m2[:], m[:], w[:],
                                                AluOpType.subtract)
                        nc.vector.tensor_scalar_add(m2[:], m2[:], 1.0)
                        m = m2
                    for ci in range(NCH):
                        cw = min(VCH, V - ci * VCH)
                        sl = slice(ci * VCH, ci * VCH + cw)
                        ot = osp.tile([128, VCH], BF16, tag="ot",
                                      name=f"ot{ci}_{tl}")
                        nc.vector.tensor_scalar_add(
                            ot[:, :cw], vsb[tl][:, sl], m[:, 0:1])
                        nc.sync.dma_start(
                            out_d.ap()[tl * 128:(tl + 1) * 128, sl],
                            ot[:, :cw],
                        )

                # tile-outer sweep: tile 0's epilogue overlaps tile 1's
                # matmuls (W_out is streamed twice; DMA has the headroom)
                for tl in range(NRTl):
                    for ci in range(min(3, NCH)):
                        emit_wo(tl, ci)
                    ro = tl * 128
                    for ci in range(NCH):
                        cw = min(VCH, V - ci * VCH)
                        sl = slice(ci * VCH, ci * VCH + cw)
                        wo = wos.pop((tl, ci))
                        wov = wo[:].rearrange("p (k n) -> p k n", n=VCH)
                        lg = lgps.tile([128, VCH], F32, tag="lg",
                                       name=f"lg{ci}_{tl}")
                        for co in range(0, cw, 512):
                            cn = min(512, cw - co)
                            # plain fp8, full 128-row stationary (DoubleRow
                            # dst is limited to partitions 0-63, and a 64-row
                            # stationary wastes half the weight stream)
                            for k in range(K_H):
                                nc.tensor.matmul(
                                    lg[:, co:co + cn],
                                    c8v[:, k, ro:ro + 128],
                                    wov[:, k, co:co + cn],
                                    start=(k == 0), stop=(k == K_H - 1),
                                )
                        # evacuate raw logits (DVE tile 0, ACT tile 1 — DVE
                        # carries tile 0's epilogue during tile 1's sweep)
                        if tl == 0:
                            nc.vector.tensor_scalar_mul(
                                vsb[0][:, sl], lg[:, :cw], INV_GS)
                        else:
                            nc.scalar.activation(vsb[1][:, sl], lg[:, :cw],
                                                 AF.Identity, scale=INV_GS)
                        junk = osbp.tile([128, VCH], F32, tag="junk",
                                         name=f"j{ci}_{tl}")
                        nc.scalar.activation(
                            junk[:, :cw], lg[:, :cw], AF.Exp, scale=INV_GS,
                            accum_out=zc[:, tl * NCH + ci:tl * NCH + ci + 1],
                        )
                        if ci + 3 < NCH:
                            emit_wo(tl, ci + 3)
                        if tl == 1 and ci == 1:
                            emit_epilogue(0)
                    if tl == NRTl - 1:
                        emit_epilogue(tl)

            import os as _os
            if _os.environ.get("KERNEL_RAW_LOGITS"):
                for tl in range(NRTl):
                    nc.sync.dma_start(out_d.ap()[tl * 128:(tl + 1) * 128, :],
                                      vsb[tl][:])


GATE_PERM = True


def _permute_gates(w):
    """PyTorch (i, f, g, o) row order -> (g, i, f, o)."""
    return np.concatenate([w[2 * H:3 * H], w[0:H], w[H:2 * H], w[3 * H:]],
                          axis=0)


def prep_inputs(features, captions, embed_table,
                W_ih1, W_hh1, b_ih1, b_hh1,
                W_ih2, W_hh2, b_ih2, b_hh2,
                W_cat, b_cat, W_out, b_out, T=32, gate_perm=GATE_PERM):
    """Host-side layout prep. Returns (common dict, per-core list, flags)."""
    import ml_dtypes
    f32 = np.float32
    bf16 = ml_dtypes.bfloat16
    fp8 = ml_dtypes.float8_e4m3
    S = np.float32(SCALE)
    features = np.asarray(features, f32)
    captions = np.asarray(captions)
    emb = np.asarray(embed_table, f32)[captions]  # [B, T, E]
    embT = np.ascontiguousarray(emb.transpose(2, 1, 0).reshape(E, T * B))  # t-major
    W_ih1, W_hh1 = np.asarray(W_ih1, f32), np.asarray(W_hh1, f32)
    W_ih2, W_hh2 = np.asarray(W_ih2, f32), np.asarray(W_hh2, f32)
    b1 = np.asarray(b_ih1, f32) + np.asarray(b_hh1, f32)
    b2 = np.asarray(b_ih2, f32) + np.asarray(b_hh2, f32)
    if gate_perm:
        W_ih1, W_hh1 = _permute_gates(W_ih1), _permute_gates(W_hh1)
        W_ih2, W_hh2 = _permute_gates(W_ih2), _permute_gates(W_hh2)
        b1, b2 = _permute_gates(b1), _permute_gates(b2)
    common = {
        "embT": (embT * S).astype(fp8),
        "wih1T": np.ascontiguousarray(W_ih1.T * S).astype(fp8),
        "whh1T": np.ascontiguousarray(W_hh1.T * S).astype(fp8),
        "wih2T": np.ascontiguousarray(W_ih2.T * S).astype(fp8),
        "whh2T": np.ascontiguousarray(W_hh2.T * S).astype(fp8),
        "wcatT2": np.ascontiguousarray(np.asarray(W_cat, f32).T).astype(bf16),
        "woutF": np.ascontiguousarray(
            np.asarray(W_out, f32).T * S).astype(fp8),
        "bcat": np.asarray(b_cat, f32),
        "b1": b1 * S * S,
        "b2": b2 * S * S,
    }
    per_core = []
    for c in range(NCORES):
        fsl = features[c * BL:(c + 1) * BL]
        sel = np.zeros((64, BL), np.float32)
        for j in range(BL):
            sel[c * BL + j, j] = 1.0
        per_core.append({
            "FTl": np.ascontiguousarray(fsl.transpose(0, 2, 1)).astype(bf16),
            "Fl": np.ascontiguousarray(fsl).astype(bf16),
            "sel": sel.astype(bf16),
        })
    bout = np.asarray(b_out, f32)
    flags = dict(
        use_b1=bool(np.any(common["b1"])),
        use_b2=bool(np.any(common["b2"])),
        use_bout=bool(np.any(bout)),
    )
    return common, per_core, flags


_NC_CACHE = {}
_EXEC_CACHE = {}
_INPUT_CACHE = {}


def _get_executor(nc, key):
    """Persistent jitted shard_map dispatcher for nc (built once per key)."""
    if key in _EXEC_CACHE:
        return _EXEC_CACHE[key]
    import jax
    from jax.sharding import Mesh, PartitionSpec, NamedSharding
    from jax.experimental.shard_map import shard_map as shard_map_fn
    import concourse.bass2jax as b2j
    import concourse.mybir as mybir_
    b2j.install_neuronx_cc_hook()

    partition_name = (nc.partition_id_tensor.name
                      if nc.partition_id_tensor else None)
    in_names, out_names, out_avals, zero_shapes = [], [], [], []
    for alloc in nc.m.functions[0].allocations:
        if not isinstance(alloc, mybir_.MemoryLocationSet):
            continue
        name = alloc.memorylocations[0].name
        if alloc.kind == "ExternalInput":
            if name != partition_name:
                in_names.append(name)
        elif alloc.kind == "ExternalOutput":
            npdt = mybir_.dt.np(alloc.dtype)
            out_names.append(name)
            out_avals.append(jax.core.ShapedArray(tuple(alloc.tensor_shape),
                                                  npdt))
            zero_shapes.append((tuple(alloc.tensor_shape), npdt))

    n_params = len(in_names)
    n_outs = len(out_names)
    all_in_names = list(in_names) + list(out_names)
    if partition_name is not None:
        all_in_names.append(partition_name)

    def _body(*args):
        operands = list(args)
        if partition_name is not None:
            operands.append(b2j.partition_id_tensor())
        outs = b2j._bass_exec_p.bind(
            *operands,
            out_avals=tuple(out_avals),
            in_names=tuple(all_in_names),
            out_names=tuple(out_names),
            lowering_input_output_aliases=(),
            sim_require_finite=True,
            sim_require_nnan=True,
            nc=nc,
        )
        return tuple(outs)

    devices = jax.devices()[:NCORES]
    mesh = Mesh(np.asarray(devices), ("core",))
    spec = PartitionSpec("core")
    in_specs = (spec,) * (n_params + n_outs)
    out_specs = (spec,) * n_outs
    sharded = jax.jit(
        shard_map_fn(_body, mesh=mesh, in_specs=in_specs,
                     out_specs=out_specs, check_rep=False),
        keep_unused=True,
    )
    sh = NamedSharding(mesh, spec)
    zeros = tuple(
        jax.device_put(np.zeros((NCORES * s[0], *s[1:]), d), sh)
        for (s, d) in zero_shapes
    )
    state = dict(sharded=sharded, in_names=in_names, out_names=out_names,
                 zeros=zeros, sh=sh)
    _EXEC_CACHE[key] = state
    return state


def _fingerprint(common, per_core):
    import hashlib
    h = hashlib.blake2b(digest_size=16)

    def upd(n, a):
        a = np.ascontiguousarray(a)
        h.update(n.encode())
        h.update(str(a.shape).encode())
        h.update(str(a.dtype).encode())
        h.update(a.tobytes())

    for n in sorted(common):
        upd(n, common[n])
    for c, pc in enumerate(per_core):
        for n in sorted(pc):
            upd(f"{c}:{n}", pc[n])
    return h.hexdigest()


def kernel(features, captions, embed_table,
           W_ih1, W_hh1, b_ih1, b_hh1,
           W_ih2, W_hh2, b_ih2, b_hh2,
           W_cat, b_cat, W_out, b_out):
    import concourse.bass2jax as b2j
    T = np.asarray(captions).shape[1]
    common, per_core, flags = prep_inputs(
        features, captions, embed_table,
        W_ih1, W_hh1, b_ih1, b_hh1,
        W_ih2, W_hh2, b_ih2, b_hh2,
        W_cat, b_cat, W_out, b_out, T=T)
    key = (T, tuple(sorted(flags.items())))
    if key not in _NC_CACHE:
        _NC_CACHE[key] = build_nc2(T=T, **flags)
    nc = _NC_CACHE[key]
    in_maps = [dict(common, **pc) for pc in per_core]
    results = b2j.run_bass_via_pjrt(nc, in_maps, n_cores=NCORES)
    out = np.concatenate([np.asarray(results[c]["out"])
                          for c in range(NCORES)], axis=0)
    # core c rows are batches [c*BL, (c+1)*BL) in b-major order already
    out = out.reshape(B, T, V_FULL)
    return np.ascontiguousarray(out).astype(np.float32)


if __name__ == "__main__":
    import time
    t0 = time.time()
    nc = build_nc2()
    print("built ok in", time.time() - t0, "s;",
          sum(len(b.instructions) for f in nc.m.functions for b in f.blocks),
          "instructions")
